# revision 1
# baseline (speedup 1.0000x reference)
"""HeightmapNormalsLoss TRN2 kernel.

Data-parallel over 8 NeuronCores: 4 image-pairs per core. Per image:
Sobel gx/gy via TensorEngine band matmuls (vertical [1,2,1]/[1,0,-1] bands
as the stationary operand, horizontal taps as shifted column streams of an
edge-padded bf16 tile), then the normal/L1 chain on DVE+ACT in bf16:

  t   = gx^2 + gy^2 + 1/63                      (DVE)
  u   = 1/t                                     (DVE RECIPROCAL_APPROX_FAST)
  inv = sqrt(16/63 * u)  = 4/sqrt(63 s + 1)     (ACT Sqrt)
  gz  = sqrt(-t/16 + 4/63) = sqrt(1-s)/4        (ACT Sqrt)
  n   = (gx, gy, gz) * inv                      (DVE)
  partial += sum |n_gen - n_tgt|                (DVE abs via abs_max + accum)

Per-core output: [128, 24] f32 partial sums; host reduces and divides.
"""
import sys

sys.path.insert(0, "/opt/trn_rl_repo")

import numpy as np
import ml_dtypes

H = W = 512
N_CORES = 8
PAIRS_PER_CORE = 4
TOTAL_B = 32

# (out_row_start, M, in_row_start, K, variant_idx)
ROW_TILES = [
    (0, 127, 0, 128, 0),
    (127, 126, 126, 128, 1),
    (253, 126, 252, 128, 1),
    (379, 126, 378, 128, 1),
    (505, 7, 504, 8, 2),
]
N_ACC_COLS = PAIRS_PER_CORE * len(ROW_TILES)  # 20


def _build_bands_np():
    """[128, 12*128] f32: blocks (band*3 + variant), bands sv, -sv, dv, 2dv."""
    mats = {}
    for v, (K, M) in enumerate([(128, 127), (128, 126), (8, 7)]):
        sv = np.zeros((128, 128), np.float32)
        dv = np.zeros((128, 128), np.float32)
        if v == 0:  # first: m=0 clamps row -1 -> 0
            sv[0, 0], sv[1, 0] = 3.0, 1.0
            dv[0, 0], dv[1, 0] = 1.0, -1.0
            for m in range(1, M):
                sv[m - 1, m], sv[m, m], sv[m + 1, m] = 1.0, 2.0, 1.0
                dv[m - 1, m], dv[m + 1, m] = 1.0, -1.0
        elif v == 1:  # mid
            for m in range(M):
                sv[m, m], sv[m + 1, m], sv[m + 2, m] = 1.0, 2.0, 1.0
                dv[m, m], dv[m + 2, m] = 1.0, -1.0
        else:  # last: m=M-1 (global 511) clamps row 512 -> 511
            for m in range(M - 1):
                sv[m, m], sv[m + 1, m], sv[m + 2, m] = 1.0, 2.0, 1.0
                dv[m, m], dv[m + 2, m] = 1.0, -1.0
            m = M - 1
            sv[m, m], sv[m + 1, m] = 1.0, 3.0
            dv[m, m], dv[m + 1, m] = 1.0, -1.0
        mats[(0, v)] = sv
        mats[(1, v)] = -sv
        mats[(2, v)] = dv
        mats[(3, v)] = 2.0 * dv
    w = np.zeros((128, 12 * 128), np.float32)
    for b in range(4):
        for v in range(3):
            w[:, (b * 3 + v) * 128 : (b * 3 + v + 1) * 128] = mats[(b, v)]
    return w.astype(ml_dtypes.bfloat16)


def _kernel_body(tc, gen_d, tgt_d, w_d, acc_d):
    from contextlib import ExitStack
    from concourse import mybir

    nc = tc.nc
    AF = mybir.ActivationFunctionType
    OP = mybir.AluOpType
    f32 = mybir.dt.float32
    bf16 = mybir.dt.bfloat16

    with ExitStack() as ctx:
        persist = ctx.enter_context(tc.tile_pool(name="persist", bufs=1))
        xp_pool = ctx.enter_context(tc.tile_pool(name="xp", bufs=6))
        ps_pool = ctx.enter_context(
            tc.tile_pool(name="ps", bufs=3, space="PSUM")
        )
        gb_pool = ctx.enter_context(tc.tile_pool(name="gb", bufs=3))
        mt_pool = ctx.enter_context(tc.tile_pool(name="mt", bufs=2))
        iv_pool = ctx.enter_context(tc.tile_pool(name="iv", bufs=2))
        nb_pool = ctx.enter_context(tc.tile_pool(name="nb", bufs=3))
        da_pool = ctx.enter_context(tc.tile_pool(name="da", bufs=2))

        wt = persist.tile([128, 12 * 128], bf16)
        nc.sync.dma_start(wt[:], w_d[:])
        accbuf = persist.tile([128, N_ACC_COLS], f32)
        nc.vector.memset(accbuf[:], 0.0)
        bias0 = persist.tile([128, 1], f32)
        nc.vector.memset(bias0[:], 0.0)
        bias_gz = persist.tile([128, 1], f32)
        nc.vector.memset(bias_gz[:], 4.0 / 63.0)

        def w_sl(band, variant, K, M):
            blk = (band * 3 + variant) * 128
            return wt[0:K, blk : blk + M]

        NT = len(ROW_TILES)  # 5
        col = 0
        for pair in range(PAIRS_PER_CORE):
            nbs = {}
            for imi, src in ((0, gen_d), (1, tgt_d)):
                # gxgy[:, rt, 0:512]=gx, [512:1024]=gy, [1024:1536]=gz (bf16)
                gxgy = gb_pool.tile([128, NT, 3 * W], bf16, tag="gxgy")
                for rt, (r0, M, i0, K, v) in enumerate(ROW_TILES):
                    xp = xp_pool.tile([128, W + 2], bf16, tag="xp")
                    # SWDGE cast-DMA: f32 DRAM -> bf16 SBUF
                    nc.gpsimd.dma_start(xp[0:K, 1 : W + 1], src[pair, i0 : i0 + K, :])
                    # replicate-pad edge columns
                    nc.vector.tensor_copy(xp[0:K, 0:1], xp[0:K, 1:2])
                    nc.vector.tensor_copy(xp[0:K, W + 1 : W + 2], xp[0:K, W : W + 1])

                    # gx -> pt[:, 0:512], gy -> pt[:, 512:1024] (2 psum banks)
                    pt = ps_pool.tile([128, 2 * W], f32, tag="pt")
                    dv = w_sl(2, v, K, M)
                    nc.tensor.matmul(
                        pt[0:M, W : 2 * W], dv, xp[0:K, 0:W], start=True, stop=False
                    )
                    nc.tensor.matmul(
                        pt[0:M, W : 2 * W], dv, xp[0:K, 2 : W + 2], start=False,
                        stop=False,
                    )
                    nc.tensor.matmul(
                        pt[0:M, W : 2 * W],
                        w_sl(3, v, K, M),
                        xp[0:K, 1 : W + 1],
                        start=False,
                        stop=True,
                    )
                    nc.tensor.matmul(
                        pt[0:M, 0:W], w_sl(0, v, K, M), xp[0:K, 0:W], start=True,
                        stop=False,
                    )
                    nc.tensor.matmul(
                        pt[0:M, 0:W],
                        w_sl(1, v, K, M),
                        xp[0:K, 2 : W + 2],
                        start=False,
                        stop=True,
                    )
                    # single extract: [M, 1024] psum -> bf16
                    nc.scalar.copy(gxgy[0:M, rt, 0 : 2 * W], pt[0:M, :])

                # batched per-image chain (junk rows beyond M are never read
                # by the final per-rt accumulation)
                m12 = mt_pool.tile([128, NT, 2 * W], bf16, tag="m12")
                nc.vector.tensor_tensor(
                    m12[:, :, :], gxgy[:, :, 0 : 2 * W], gxgy[:, :, 0 : 2 * W], OP.mult
                )
                t3 = mt_pool.tile([128, NT, W], bf16, tag="t3")
                nc.vector.tensor_tensor(
                    t3[:, :, :], m12[:, :, 0:W], m12[:, :, W : 2 * W], OP.add
                )
                nc.vector.tensor_scalar(
                    t3[:, :, :], t3[:, :, :], 1.0 / 63.0, None, OP.add
                )
                u3 = mt_pool.tile([128, NT, W], bf16, tag="u3")
                from concourse.dve_ops import (
                    RECIPROCAL_APPROX_FAST,
                    RECIP_APPROX_FAST_CONSTS,
                )

                nc.vector._custom_dve(
                    RECIPROCAL_APPROX_FAST,
                    out=u3[:, :, :],
                    in0=t3[:, :, :],
                    **RECIP_APPROX_FAST_CONSTS,
                )
                inv3 = iv_pool.tile([128, NT, W], bf16, tag="inv3")
                nc.scalar.activation(
                    inv3[:, :, :], u3[:, :, :], AF.Sqrt, bias=bias0[:, :],
                    scale=16.0 / 63.0,
                )
                nc.scalar.activation(
                    gxgy[:, :, 2 * W : 3 * W], t3[:, :, :], AF.Sqrt,
                    bias=bias_gz[:, :], scale=-1.0 / 16.0,
                )

                nb = nb_pool.tile([128, NT, 3 * W], bf16, tag="nb")
                for ch in range(3):
                    nc.vector.tensor_tensor(
                        nb[:, :, ch * W : (ch + 1) * W],
                        gxgy[:, :, ch * W : (ch + 1) * W],
                        inv3[:, :, :],
                        OP.mult,
                    )
                nbs[imi] = nb

            dd = da_pool.tile([128, NT, 3 * W], bf16, tag="d")
            nc.vector.tensor_tensor(
                dd[:, :, :], nbs[0][:, :, :], nbs[1][:, :, :], OP.subtract
            )
            for rt, (r0, M, i0, K, v) in enumerate(ROW_TILES):
                a = da_pool.tile([128, 3 * W], bf16, tag="a")
                nc.scalar.activation(
                    a[0:M, :], dd[0:M, rt, :], AF.Abs, bias=bias0[0:M, :],
                    accum_out=accbuf[0:M, col : col + 1],
                )
                col += 1

        nc.sync.dma_start(acc_d[:], accbuf[:])


_CACHE = {}


def _get_module():
    if "nc" not in _CACHE:
        from concourse import bacc, tile, mybir

        nc = bacc.Bacc(
            "TRN2",
            target_bir_lowering=False,
            debug=False,
            enable_asserts=True,
            num_devices=N_CORES,
        )
        gen_d = nc.dram_tensor(
            "gen", (PAIRS_PER_CORE, H, W), mybir.dt.float32, kind="ExternalInput"
        ).ap()
        tgt_d = nc.dram_tensor(
            "tgt", (PAIRS_PER_CORE, H, W), mybir.dt.float32, kind="ExternalInput"
        ).ap()
        w_d = nc.dram_tensor(
            "w", (128, 12 * 128), mybir.dt.bfloat16, kind="ExternalInput"
        ).ap()
        acc_d = nc.dram_tensor(
            "acc", (128, N_ACC_COLS), mybir.dt.float32, kind="ExternalOutput"
        ).ap()
        with tile.TileContext(nc) as tc:
            _kernel_body(tc, gen_d, tgt_d, w_d, acc_d)
        nc.compile()
        _CACHE["nc"] = nc
        _CACHE["w"] = _build_bands_np()
    return _CACHE["nc"], _CACHE["w"]


def _run(generated, target, **spmd_kwargs):
    from concourse import bass_utils

    nc, w = _get_module()
    g = np.ascontiguousarray(np.asarray(generated, np.float32).reshape(TOTAL_B, H, W))
    t = np.ascontiguousarray(np.asarray(target, np.float32).reshape(TOTAL_B, H, W))
    in_maps = [
        {
            "gen": g[c * PAIRS_PER_CORE : (c + 1) * PAIRS_PER_CORE],
            "tgt": t[c * PAIRS_PER_CORE : (c + 1) * PAIRS_PER_CORE],
            "w": w,
        }
        for c in range(N_CORES)
    ]
    return bass_utils.run_bass_kernel_spmd(
        nc, in_maps, core_ids=list(range(N_CORES)), **spmd_kwargs
    )


def kernel(generated, target):
    res = _run(generated, target)
    total = 0.0
    for r in res.results:
        total += float(np.asarray(r["acc"], np.float64).sum())
    return np.float32(total / (TOTAL_B * 3 * H * W))



# revision 8
# speedup vs baseline: 1.0882x; 1.0882x over previous
"""HeightmapNormalsLoss TRN2 kernel.

Data-parallel over 8 NeuronCores: 4 image-pairs per core. Per pair:
Sobel X,Y via TensorEngine band matmuls (vertical bands stationary, x4
scale folded into the weights, gen+tgt batched in one moving operand),
then per image:

  s"  = X^2 + Y^2                  (custom DVE op, X=4gx)
  r   = Rsqrt(63/16 s" + 1)        (ACT reciprocal_sqrt table)
  nx  = X*r, ny = Y*r              (DVE TT, bf16 2x mode)
  nz  = (1 - s"(s"/2048 + 1/32))*r (custom DVE op; deg-2 poly == sqrt(1-s))
  dd  = n_gen - n_tgt              (DVE TT)
  acc = sum |dd|                   (ACT Abs + accum)

Stationary blocks are zero-padded to 128 output rows so junk PSUM rows
are exactly 0 -> r=1, nz=1 on both images -> |dd|=0 (junk-safe, no
masking). Per-core output: [128, 4] f32 partial sums; host reduces.
"""
import sys

sys.path.insert(0, "/opt/trn_rl_repo")

import numpy as np
import ml_dtypes

H = W = 512
N_CORES = 8
PAIRS_PER_CORE = 4
TOTAL_B = 32
NT = 5

# (out_row_start, M, in_row_start, K, variant_idx)
ROW_TILES = [
    (0, 127, 0, 128, 0),
    (127, 126, 126, 128, 1),
    (253, 126, 252, 128, 1),
    (379, 126, 378, 128, 1),
    (505, 7, 504, 8, 2),
]


def _build_bands_np():
    """[128, 12*128] f32: blocks (band*3 + variant), bands sv, -sv, dv, 2dv.
    All entries x4 (folds the 4/len normalization into the matmul)."""
    mats = {}
    for v, (K, M) in enumerate([(128, 127), (128, 126), (8, 7)]):
        sv = np.zeros((128, 128), np.float32)
        dv = np.zeros((128, 128), np.float32)
        if v == 0:  # first: m=0 clamps row -1 -> 0
            sv[0, 0], sv[1, 0] = 3.0, 1.0
            dv[0, 0], dv[1, 0] = 1.0, -1.0
            for m in range(1, M):
                sv[m - 1, m], sv[m, m], sv[m + 1, m] = 1.0, 2.0, 1.0
                dv[m - 1, m], dv[m + 1, m] = 1.0, -1.0
        elif v == 1:  # mid
            for m in range(M):
                sv[m, m], sv[m + 1, m], sv[m + 2, m] = 1.0, 2.0, 1.0
                dv[m, m], dv[m + 2, m] = 1.0, -1.0
        else:  # last: m=M-1 (global 511) clamps row 512 -> 511
            for m in range(M - 1):
                sv[m, m], sv[m + 1, m], sv[m + 2, m] = 1.0, 2.0, 1.0
                dv[m, m], dv[m + 2, m] = 1.0, -1.0
            m = M - 1
            sv[m, m], sv[m + 1, m] = 1.0, 3.0
            dv[m, m], dv[m + 1, m] = 1.0, -1.0
        mats[(0, v)] = 4.0 * sv
        mats[(1, v)] = -4.0 * sv
        mats[(2, v)] = 4.0 * dv
        mats[(3, v)] = 8.0 * dv
    w = np.zeros((128, 12 * 128), np.float32)
    for b in range(4):
        for v in range(3):
            w[:, (b * 3 + v) * 128 : (b * 3 + v + 1) * 128] = mats[(b, v)]
    return w.astype(ml_dtypes.bfloat16)


_REG = {}


def _get_custom_ops():
    """Register the two fused DVE ops (once per process) and return them."""
    if _REG:
        return _REG
    from concourse import dve_ops as DO
    from concourse.dve_spec import Spec, Src0, Src1, C0, C1, C2, lower, sq, _has_src1
    from concourse.dve_uop import DveOpSpec

    def ref_sqsum(in0, in1, c0, c1, c2):
        return in0.astype(np.float32) ** 2 + in1.astype(np.float32) ** 2

    def ref_nzpr(in0, in1, c0, c1, c2):
        s = in0.astype(np.float32)
        r = in1.astype(np.float32)
        return r - s * (s * c0 + c1) * r

    defs = [
        ("ANT_SQSUM_HN", Spec(body=sq(Src0) + sq(Src1), reference=ref_sqsum)),
        (
            "ANT_NZPR_HN",
            Spec(body=Src1 - (Src0 * (Src0 * C0 + C1)) * Src1, reference=ref_nzpr),
        ),
    ]
    for name, spec in defs:
        if name not in DO._SUB_OPCODE_FOR_NAME:
            row = DO._CUSTOM_DVE_ROW_BASE + len(DO.OPS)
            DO._SUB_OPCODE_FOR_NAME[name] = row
            shas = {}
            for ver in ("v3", "v4"):
                uops = lower(spec, ver=ver)
                shas[ver] = DveOpSpec(
                    name=name, opcode=row, uops=uops, rd1_en=_has_src1(spec)
                ).sha(ver)
            op = DO.DveOp(name, spec, subdim=False, uops_sha=shas)
            DO.OPS.append(op)
            DO.CUSTOM_DVE_SPECS[name] = spec
        _REG[name] = next(o for o in DO.OPS if o.name == name)
    return _REG


def _kernel_body(tc, gen_d, tgt_d, w_d, acc_d):
    from contextlib import ExitStack
    from concourse import mybir

    ops = _get_custom_ops()
    nc = tc.nc
    AF = mybir.ActivationFunctionType
    OP = mybir.AluOpType
    f32 = mybir.dt.float32
    bf16 = mybir.dt.bfloat16

    with ExitStack() as ctx:
        persist = ctx.enter_context(tc.tile_pool(name="persist", bufs=1))
        xp_pool = ctx.enter_context(tc.tile_pool(name="xp", bufs=4))
        ps_pool = ctx.enter_context(tc.tile_pool(name="ps", bufs=2, space="PSUM"))
        c_pool = ctx.enter_context(tc.tile_pool(name="c", bufs=2))
        s_pool = ctx.enter_context(tc.tile_pool(name="s", bufs=2))
        r_pool = ctx.enter_context(tc.tile_pool(name="r", bufs=2))
        n_pool = ctx.enter_context(tc.tile_pool(name="n", bufs=2))
        d_pool = ctx.enter_context(tc.tile_pool(name="d", bufs=2))
        a_pool = ctx.enter_context(tc.tile_pool(name="a", bufs=1))

        wt = persist.tile([128, 12 * 128], bf16)
        nc.sync.dma_start(wt[:], w_d[:])
        accbuf = persist.tile([128, PAIRS_PER_CORE], f32)
        nc.vector.memset(accbuf[:], 0.0)
        bias0 = persist.tile([128, 1], f32)
        nc.vector.memset(bias0[:], 0.0)
        bias1 = persist.tile([128, 1], f32)
        nc.vector.memset(bias1[:], 1.0)

        def w_sl(band, v, K):
            blk = (band * 3 + v) * 128
            return wt[0:K, blk : blk + 128]

        for pair in range(PAIRS_PER_CORE):
            # per-image extracted Sobel responses: [row, rt, X(512)|Y(512)]
            cc = [
                c_pool.tile([128, NT, 2 * W], bf16, tag=f"c{im}", name=f"c{im}")
                for im in range(2)
            ]
            for rt, (r0, M, i0, K, v) in enumerate(ROW_TILES):
                xp = xp_pool.tile([128, 2, W + 2], bf16, tag="xp")
                nc.sync.dma_start(xp[0:K, 0, 1 : W + 1], gen_d[pair, i0 : i0 + K, :])
                nc.sync.dma_start(xp[0:K, 1, 1 : W + 1], tgt_d[pair, i0 : i0 + K, :])
                # replicate-pad edge columns (both images in one op)
                nc.gpsimd.tensor_copy(xp[0:K, :, 0:1], xp[0:K, :, 1:2])
                nc.gpsimd.tensor_copy(xp[0:K, :, W + 1 : W + 2], xp[0:K, :, W : W + 1])

                # pt: [128, img, X|Y]; full 128 out rows (junk rows = 0).
                # 2D matmuls ordered by stationary weight (one load each);
                # (band, col_shift, start, stop) per psum accumulation group.
                pt = ps_pool.tile([128, 2, 2 * W], f32, tag="pt")
                plan = [
                    (2, 0, W, True, False),   # Y += dv @ left
                    (2, 2, W, False, False),  # Y += dv @ right
                    (3, 1, W, False, True),   # Y += 2dv @ mid
                    (0, 0, 0, True, False),   # X += sv @ left
                    (1, 2, 0, False, True),   # X += -sv @ right
                ]
                for band, sh, co, st, sp in plan:
                    ws = w_sl(band, v, K)
                    for im in range(2):
                        nc.tensor.matmul(
                            pt[:, im, co : co + W],
                            ws,
                            xp[0:K, im, sh : sh + W],
                            start=st,
                            stop=sp,
                        )
                # extract psum f32 -> sbuf bf16, one op per image
                nc.scalar.copy(cc[0][:, rt, :], pt[:, 0, :])
                nc.scalar.copy(cc[1][:, rt, :], pt[:, 1, :])

            nn = []  # per image: (nx|ny tile, nz tile)
            for im in range(2):
                cx = cc[im][:, :, 0:W]
                cy = cc[im][:, :, W : 2 * W]
                s2 = s_pool.tile([128, NT, W], bf16, tag=f"s2{im}")
                nc.vector._custom_dve(
                    ops["ANT_SQSUM_HN"], out=s2[:], in0=cx, in1=cy
                )
                rr = r_pool.tile([128, NT, W], bf16, tag=f"rr{im}")
                nc.scalar.activation(
                    rr[:],
                    s2[:],
                    AF.Abs_reciprocal_sqrt,
                    bias=bias1[:, :],
                    scale=63.0 / 16.0,
                )
                nxy = n_pool.tile([128, NT, 2 * W], bf16, tag=f"nxy{im}")
                nc.vector.tensor_tensor(nxy[:, :, 0:W], cx, rr[:], OP.mult)
                nc.vector.tensor_tensor(nxy[:, :, W : 2 * W], cy, rr[:], OP.mult)
                nz = n_pool.tile([128, NT, W], bf16, tag=f"nz{im}")
                nc.vector._custom_dve(
                    ops["ANT_NZPR_HN"],
                    out=nz[:],
                    in0=s2[:],
                    in1=rr[:],
                    s0=1.0 / 2048.0,
                    s1=1.0 / 32.0,
                )
                nn.append((nxy, nz))

            dt = d_pool.tile([128, NT, 3 * W], bf16, tag="dt")
            nc.vector.tensor_tensor(
                dt[:, :, 0 : 2 * W], nn[0][0][:], nn[1][0][:], OP.subtract
            )
            nc.vector.tensor_tensor(
                dt[:, :, 2 * W : 3 * W], nn[0][1][:], nn[1][1][:], OP.subtract
            )
            at = a_pool.tile([128, NT, 3 * W], bf16, tag="at")
            nc.scalar.activation(
                at[:],
                dt[:],
                AF.Abs,
                bias=bias0[:, :],
                accum_out=accbuf[:, pair : pair + 1],
            )

        nc.sync.dma_start(acc_d[:], accbuf[:])


_CACHE = {}


def _get_module():
    if "nc" not in _CACHE:
        from concourse import bacc, tile, mybir

        nc = bacc.Bacc(
            "TRN2",
            target_bir_lowering=False,
            debug=False,
            enable_asserts=True,
            num_devices=N_CORES,
        )
        gen_d = nc.dram_tensor(
            "gen", (PAIRS_PER_CORE, H, W), mybir.dt.bfloat16, kind="ExternalInput"
        ).ap()
        tgt_d = nc.dram_tensor(
            "tgt", (PAIRS_PER_CORE, H, W), mybir.dt.bfloat16, kind="ExternalInput"
        ).ap()
        w_d = nc.dram_tensor(
            "w", (128, 12 * 128), mybir.dt.bfloat16, kind="ExternalInput"
        ).ap()
        acc_d = nc.dram_tensor(
            "acc", (128, PAIRS_PER_CORE), mybir.dt.float32, kind="ExternalOutput"
        ).ap()
        with tile.TileContext(nc) as tc:
            _kernel_body(tc, gen_d, tgt_d, w_d, acc_d)
        nc.compile()
        _CACHE["nc"] = nc
        _CACHE["w"] = _build_bands_np()
    return _CACHE["nc"], _CACHE["w"]


def _run(generated, target, **spmd_kwargs):
    from concourse import bass_utils

    nc, w = _get_module()
    g = np.asarray(generated, np.float32).reshape(TOTAL_B, H, W)
    t = np.asarray(target, np.float32).reshape(TOTAL_B, H, W)
    g = np.ascontiguousarray(g).astype(ml_dtypes.bfloat16)
    t = np.ascontiguousarray(t).astype(ml_dtypes.bfloat16)
    in_maps = [
        {
            "gen": g[c * PAIRS_PER_CORE : (c + 1) * PAIRS_PER_CORE],
            "tgt": t[c * PAIRS_PER_CORE : (c + 1) * PAIRS_PER_CORE],
            "w": w,
        }
        for c in range(N_CORES)
    ]
    return bass_utils.run_bass_kernel_spmd(
        nc, in_maps, core_ids=list(range(N_CORES)), **spmd_kwargs
    )


def kernel(generated, target):
    res = _run(generated, target)
    total = 0.0
    for r in res.results:
        total += float(np.asarray(r["acc"], np.float64).sum())
    return np.float32(total / (TOTAL_B * 3 * H * W))
